# revision 9
# baseline (speedup 1.0000x reference)
"""Masked spatial RMSE loss on 8 trn2 NeuronCores — 2x4 sharded variant.

Same math as kernel.py (V = m @ (W+I), reduce against sq), but the core grid
is 2 i-halves x 4 n-quarters, so the replicated mask costs 4MB/core instead
of 8MB, at the price of fp8 y/yhat slices twice as wide.  Per-core HBM:
mask 4MB + W' 2MB + y/yh 2+2MB = 10MB.  Cores with the same i-half carry
identical mask data; their count reduction is disambiguated by rolling the
inputs along b by -(n_quarter*32) rows so the four replicas count disjoint
row groups.
"""

import numpy as np

B = 2048
N = 4096
NCORES = 8
CI = 2           # i-halves
CN = 4           # n-quarters
IH = N // CI     # 2048 contraction rows per core
NQ = N // CN     # 1024 output columns per core
P = 128
T = B // P       # 16 b-tiles
IC = IH // P     # 16 i-chunks per core
GP = IC // 2     # 8 DoubleRow pairs
NH = NQ // 512   # 2 psum tiles per b-tile
EPS = 1e-6
WSCALE = 1.25

_CACHE: dict = {}


def build_program(repeat=1, nwarm=10):
    import concourse.bass as bass  # noqa: F401
    import concourse.tile as tile
    from concourse import bacc, mybir

    f32 = mybir.dt.float32
    f8 = mybir.dt.float8e4
    Alu = mybir.AluOpType
    Act = mybir.ActivationFunctionType
    DR = mybir.MatmulPerfMode.DoubleRow

    nc = bacc.Bacc(
        "TRN2", target_bir_lowering=False, debug=False, num_devices=NCORES
    )

    yhat_d = nc.dram_tensor("yhat_s", [B, NQ], f8, kind="ExternalInput").ap()
    y_d = nc.dram_tensor("y_s", [B, NQ], f8, kind="ExternalInput").ap()
    mt_d = nc.dram_tensor("mt", [B, IH], f8, kind="ExternalInput").ap()
    wps_d = nc.dram_tensor("wps", [IH, NQ], f8, kind="ExternalInput").ap()
    out_d = nc.dram_tensor("out", [P, 4], f32, kind="ExternalOutput").ap()

    yhat_v = yhat_d.rearrange("(t p) n -> p t n", p=P)  # [128, 16, 1024]
    y_v = y_d.rearrange("(t p) n -> p t n", p=P)
    # host pre-packed: mt[t*128 + p, c*128 + j] = mask[t*128 + j, ihalf: c*128 + p]
    mt_v = mt_d.rearrange("(t p) (c j) -> p t c j", p=P, c=IC)
    wps_v = wps_d.rearrange("(c p) n -> p c n", p=P)  # [128, 16, 1024]

    with tile.TileContext(nc) as tc:
        with (
            tc.tile_pool(name="persist", bufs=1) as persist,
            tc.tile_pool(name="mtp", bufs=16) as mtp,
            tc.tile_pool(name="io", bufs=3) as iop,
            tc.tile_pool(name="scratch", bufs=2) as scratch,
            tc.tile_pool(name="psum", bufs=8, space="PSUM") as psum,
        ):
            for rep in range(repeat):
                sq_f8 = persist.tile(
                    [P, T, NQ], f8, tag="sq", bufs=2, name=f"sq{rep}"
                )
                wps_sb = persist.tile(
                    [P, IC, NQ], f8, tag="wps", bufs=2, name=f"wps{rep}"
                )
                acc = persist.tile(
                    [P, T * NH], f32, tag="acc", bufs=2, name=f"acc{rep}"
                )
                cntc = persist.tile([P, T], f32, tag="cnt", bufs=2, name=f"cnt{rep}")

                for w_ in range(nwarm if rep == 0 else 0):
                    if w_ == 0:
                        dum = persist.tile(
                            [P, 2, 512], f8, tag="dum", bufs=1, name=f"dum{rep}"
                        )
                        nc.vector.memset(dum, 0.0)
                    ps_w = psum.tile(
                        [P, 512], f32, tag="ps", name=f"psw{rep}_{w_}"
                    )
                    nc.tensor.matmul(
                        ps_w,
                        lhsT=dum[:, :, 0:P],
                        rhs=dum,
                        start=True,
                        stop=True,
                        perf_mode=DR,
                    )

                mts = [None] * T
                yhp = [None] * (T // 2)
                yyp = [None] * (T // 2)

                def dma_wq(q):
                    nc.sync.dma_start(
                        out=wps_sb[:, 4 * q : 4 * (q + 1), :],
                        in_=wps_v[:, 4 * q : 4 * (q + 1), :],
                    )

                def dma_mt(t):
                    mts[t] = mtp.tile(
                        [P, IC, P], f8, tag="mt", name=f"mt{rep}_{t}"
                    )
                    nc.sync.dma_start(out=mts[t], in_=mt_v[:, t])

                def dma_piece(j):
                    yhp[j] = iop.tile(
                        [P, 2, NQ], f8, tag="yh", name=f"yh{rep}_{j}"
                    )
                    yyp[j] = iop.tile(
                        [P, 2, NQ], f8, tag="yy", name=f"yy{rep}_{j}"
                    )
                    sl = slice(2 * j, 2 * j + 2)
                    nc.sync.dma_start(out=yhp[j], in_=yhat_v[:, sl, :])
                    nc.sync.dma_start(out=yyp[j], in_=y_v[:, sl, :])

                dma_wq(0)
                dma_mt(0)
                dma_wq(1)
                dma_mt(1)
                dma_wq(2)
                dma_wq(3)
                dma_piece(0)
                dma_mt(2)
                dma_piece(1)
                dma_mt(3)
                dma_piece(2)
                dma_mt(4)
                dma_piece(3)
                dma_mt(5)
                dma_mt(6)
                dma_piece(4)
                dma_mt(7)
                dma_mt(8)
                dma_piece(5)
                dma_mt(9)
                dma_mt(10)
                dma_piece(6)
                dma_mt(11)
                dma_mt(12)
                dma_piece(7)
                dma_mt(13)
                dma_mt(14)
                dma_mt(15)

                for t in range(T):
                    if t % 2 == 0:
                        j = t // 2
                        d_t = scratch.tile(
                            [P, 2, NQ],
                            mybir.dt.bfloat16,
                            tag="d",
                            name=f"d{rep}_{j}",
                        )
                        nc.vector.tensor_sub(d_t, yyp[j], yhp[j])
                        nc.scalar.activation(
                            sq_f8[:, 2 * j : 2 * j + 2, :], d_t, Act.Square
                        )
                    for nh in range(NH):
                        ps_t = psum.tile(
                            [P, 512], f32, tag="ps", name=f"ps{rep}_{t}_{nh}"
                        )
                        nsl = slice(nh * 512, (nh + 1) * 512)
                        for g in range(GP):
                            nc.tensor.matmul(
                                ps_t,
                                lhsT=mts[t][:, 2 * g : 2 * g + 2, :],
                                rhs=wps_sb[:, 2 * g : 2 * g + 2, nsl],
                                start=(g == 0),
                                stop=(g == GP - 1),
                                perf_mode=DR,
                            )
                        tr = scratch.tile(
                            [P, 512], f32, tag="tr", name=f"tr{rep}_{t}_{nh}"
                        )
                        k = t * NH + nh
                        nc.vector.scalar_tensor_tensor(
                            out=tr,
                            in0=ps_t,
                            scalar=1.0,
                            in1=sq_f8[:, t, nsl],
                            op0=Alu.mult,
                            op1=Alu.mult,
                            accum_out=acc[:, k : k + 1],
                        )
                    ct = scratch.tile(
                        [P, IC, 32], mybir.dt.bfloat16, tag="ct",
                        name=f"ct{rep}_{t}"
                    )
                    nc.scalar.activation(
                        ct,
                        mts[t][:, :, 0:32],
                        Act.Copy,
                        accum_out=cntc[:, t : t + 1],
                    )

                out_sb = persist.tile([P, 4], f32, tag="os", bufs=2, name=f"os{rep}")
                nc.vector.memset(out_sb, 0.0)
                nc.vector.tensor_reduce(
                    out=out_sb[:, 0:1], in_=acc, axis=mybir.AxisListType.X,
                    op=Alu.add
                )
                nc.vector.tensor_reduce(
                    out=out_sb[:, 2:3], in_=cntc, axis=mybir.AxisListType.X,
                    op=Alu.add
                )
                nc.sync.dma_start(out=out_d, in_=out_sb)

    nc.compile()
    return nc


def make_in_maps(yhat, y, batch_mask, spots_neighbors):
    import ml_dtypes

    f8 = ml_dtypes.float8_e4m3

    mask_u8 = (np.ascontiguousarray(batch_mask) != 0).astype(np.uint8)
    yhat = np.ascontiguousarray(yhat, dtype=np.float32)
    y = np.ascontiguousarray(y, dtype=np.float32)
    w = np.ascontiguousarray(spots_neighbors, dtype=np.float32)

    in_maps = []
    for c in range(NCORES):
        ih, nq = divmod(c, CN)
        isl = slice(ih * IH, (ih + 1) * IH)
        nsl = slice(nq * NQ, (nq + 1) * NQ)
        roll = -(nq * 32)  # 4 replicas of each i-half count disjoint b rows
        wps = w[isl, nsl] * WSCALE
        # diagonal of W+I restricted to this (i-half, n-quarter) block
        gn = np.arange(nq * NQ, (nq + 1) * NQ)
        hit = (gn >= ih * IH) & (gn < (ih + 1) * IH)
        wps[gn[hit] - ih * IH, np.nonzero(hit)[0]] += WSCALE
        wps8 = wps.astype(f8)
        mrow = np.roll(mask_u8, roll, axis=0)
        yh = np.roll(yhat[:, nsl], roll, axis=0).astype(f8)
        yy = np.roll(y[:, nsl], roll, axis=0).astype(f8)
        m8 = (mrow[:, isl] * 0x38).astype(np.uint8)
        mt = (
            m8.reshape(T, P, IC, P)
            .transpose(0, 3, 2, 1)
            .reshape(B, IH)
        )
        mt = np.ascontiguousarray(mt).view(f8)
        in_maps.append(
            {
                "yhat_s": np.ascontiguousarray(yh),
                "y_s": np.ascontiguousarray(yy),
                "mt": mt,
                "wps": np.ascontiguousarray(wps8),
            }
        )
    return in_maps


def combine_outs(outs):
    s = 0.0
    cnt = 0.0
    for o in outs:
        o64 = o.astype(np.float64)
        s += o64[:, 0].sum()
        cnt += o64[:, 2].sum()
    loss = np.sqrt(s / WSCALE / cnt + EPS)
    return np.array(loss, dtype=np.float32)


def kernel(yhat, y, batch_mask, spots_neighbors):
    from concourse.bass_utils import run_bass_kernel_spmd

    if "nc" not in _CACHE:
        _CACHE["nc"] = build_program()
    nc = _CACHE["nc"]
    in_maps = make_in_maps(yhat, y, batch_mask, spots_neighbors)
    res = run_bass_kernel_spmd(nc, in_maps, list(range(NCORES))).results
    return combine_outs([res[c]["out"] for c in range(NCORES)])
